# revision 1
# baseline (speedup 1.0000x reference)
"""Causal attention kernel for Trainium2, 8-core SPMD.

Problem: B=4, N=4096, D_IN=D_OUT=1024 single-head causal attention
    q,k,v = x@Wq.T, x@Wk.T, x@Wv.T ; out = softmax(mask(q k^T/32)) v

Sharding: 8 cores = 4 batches x 2 output-column halves (tensor-parallel
on Wv columns).  Every core computes the full Q/K projections and all
N x N causal scores for its batch, but only its 512 output columns of
V / attn@V.  No collectives; host divides by the softmax denominator
(each core also outputs l = sum_k exp(s)) and concatenates.

Dataflow (per core, all f32r matmuls at 1 cycle/row):
  phase 1: K^T[o,n] and V[n,ov] projections -> DRAM scratch
  phase 2: per 512-query chunk: fused Q^T projection, then stream
    K^T/V tiles; S^T[k,q] = K^T.T @ Q^T in PSUM (accumulate over o);
    causal handled by skipping fully-masked k-tiles + 4 static masks
    on the diagonal tiles; P = exp(S/32) via ACT (no max subtraction:
    |scores| < ~3 by construction); AV += P^T.T @ V accumulates in
    PSUM across the whole k extent; l via ones-matmul.
"""
import numpy as np

import concourse.bacc as bacc
import concourse.mybir as mybir
import concourse.tile as tile
from concourse.bass_utils import run_bass_kernel_spmd

F32 = mybir.dt.float32
F32R = mybir.dt.float32r
EXP = mybir.ActivationFunctionType.Exp

B, N, E, O, OV = 4, 4096, 1024, 1024, 512
NE, NO = E // 128, O // 128          # 8 subtiles on contraction dims
QC = 512                             # query chunk
NCHUNK = N // QC                     # 8
NKT = N // 128                       # 32 k-tiles
NH = N // 2                          # phase-1 n half
SCALE = 1.0 / 32.0                   # 1/sqrt(D_OUT)
MASKVAL = -1e9


def _emit(nc, tc, xT, WqT, WkT, WvT, KT_d, V_d, out_u, l_out):
    xT3 = xT.rearrange("(a p) n -> a p n", p=128)
    WqT3 = WqT.rearrange("(a p) n -> a p n", p=128)
    WkT3 = WkT.rearrange("(a p) n -> a p n", p=128)
    WvT3 = WvT.rearrange("(a p) n -> a p n", p=128)
    # K^T viewed so one DMA grabs [o=1024, k=128] as a [128, 8, 128] tile
    KT_v = KT_d.rearrange("(a p) n -> p a n", p=128)

    # ---------------- phase 1: K^T and V projections ----------------
    with tc.tile_pool(name="p1", bufs=1) as sb, \
         tc.tile_pool(name="p1p", bufs=1, space="PSUM") as pp:
        wk = []
        wv = []
        for e in range(NE):
            wkt = sb.tile([128, O], F32R, tag=f"wk{e}", name=f"wk_{e}")
            nc.sync.dma_start(wkt, WkT3[e].bitcast(F32R))
            wk.append(wkt)
            wvt = sb.tile([128, OV], F32R, tag=f"wv{e}", name=f"wv_{e}")
            nc.sync.dma_start(wvt, WvT3[e].bitcast(F32R))
            wv.append(wvt)
        for half in range(2):
            xe = []
            for e in range(NE):
                xet = sb.tile([128, NH], F32R, tag=f"xe{e}", name=f"xe_{half}_{e}")
                nc.sync.dma_start(
                    xet, xT3[e][:, half * NH:(half + 1) * NH].bitcast(F32R))
                xe.append(xet)
            for o in range(NO):
                pk = [pp.tile([128, 512], F32, tag="pp", bufs=8,
                              name=f"pk_{half}_{o}_{kc}") for kc in range(4)]
                for e in range(NE):
                    for kc in range(4):
                        nc.tensor.matmul(
                            pk[kc], wk[e][:, o * 128:(o + 1) * 128],
                            xe[e][:, kc * 512:(kc + 1) * 512],
                            start=(e == 0), stop=(e == NE - 1))
                for kc in range(4):
                    ksb = sb.tile([128, 512], F32, tag="ksb", bufs=3,
                                  name=f"ksb_{half}_{o}_{kc}")
                    nc.scalar.copy(ksb, pk[kc])
                    nc.sync.dma_start(
                        KT_d[o * 128:(o + 1) * 128,
                             half * NH + kc * 512: half * NH + (kc + 1) * 512],
                        ksb)
            for ns in range(NH // 128):
                pv = pp.tile([128, OV], F32, tag="pp", bufs=8,
                             name=f"pv_{half}_{ns}")
                for e in range(NE):
                    nc.tensor.matmul(
                        pv, xe[e][:, ns * 128:(ns + 1) * 128], wv[e],
                        start=(e == 0), stop=(e == NE - 1))
                vsb = sb.tile([128, OV], F32, tag="vsb", bufs=3,
                              name=f"vsb_{half}_{ns}")
                nc.scalar.copy(vsb, pv)
                row = half * NH + ns * 128
                nc.sync.dma_start(V_d[row:row + 128, :], vsb)

    # ---------------- phase 2: attention ----------------
    with tc.tile_pool(name="p2", bufs=1) as sb, \
         tc.tile_pool(name="p2p", bufs=1, space="PSUM") as pp:
        # static diagonal masks: mask[i][kk, qq] = 0 if qq >= i*128+kk else MASKVAL
        masks = []
        for i in range(4):
            m = sb.tile([128, QC], F32, tag=f"mask{i}", name=f"mask_{i}")
            nc.gpsimd.memset(m, 0.0)
            nc.gpsimd.affine_select(
                out=m, in_=m, compare_op=mybir.AluOpType.is_ge,
                fill=MASKVAL, base=-i * 128, pattern=[[1, QC]],
                channel_multiplier=-1)
            masks.append(m)
        ones_f = sb.tile([128, 1], F32, tag="ones_f", name="ones_f")
        nc.gpsimd.memset(ones_f, 1.0)
        ones = sb.tile([128, 1], F32R, tag="ones", name="ones")
        nc.scalar.copy(ones, ones_f)

        wq = []
        for e in range(NE):
            wqt = sb.tile([128, O], F32R, tag=f"wq{e}", name=f"wq_{e}")
            nc.sync.dma_start(wqt, WqT3[e].bitcast(F32R))
            wq.append(wqt)

        out3 = out_u.rearrange("(a p) n -> a p n", p=128)

        for c in range(NCHUNK):
            nkt = 4 * (c + 1)
            # fused Q^T projection for this chunk
            xc = []
            for e in range(NE):
                xct = sb.tile([128, QC], F32R, tag=f"xc{e}", bufs=2,
                              name=f"xc_{c}_{e}")
                nc.sync.dma_start(
                    xct, xT3[e][:, c * QC:(c + 1) * QC].bitcast(F32R))
                xc.append(xct)
            qt = []
            for oi in range(NO):
                qps = pp.tile([128, QC], F32, tag="avqp", bufs=5,
                              name=f"qps_{c}_{oi}")
                for e in range(NE):
                    nc.tensor.matmul(
                        qps, wq[e][:, oi * 128:(oi + 1) * 128], xc[e],
                        start=(e == 0), stop=(e == NE - 1))
                qtt = sb.tile([128, QC], F32R, tag=f"qt{oi}", bufs=2,
                              name=f"qt_{c}_{oi}")
                nc.scalar.copy(qtt, qps)
                qt.append(qtt)

            av = [pp.tile([128, OV], F32, tag="avqp", bufs=5,
                          name=f"av_{c}_{s}") for s in range(4)]
            lps = pp.tile([1, QC], F32, tag="l", bufs=1, name=f"lps_{c}")

            for kt in range(nkt):
                ktile = sb.tile([128, NO, 128], F32R, tag="ktile", bufs=6,
                                name=f"ktile_{c}_{kt}")
                nc.sync.dma_start(
                    ktile, KT_v[:, :, kt * 128:(kt + 1) * 128].bitcast(F32R))
                vt = sb.tile([128, OV], F32R, tag="vt", bufs=6,
                             name=f"vt_{c}_{kt}")
                nc.sync.dma_start(
                    vt, V_d[kt * 128:(kt + 1) * 128, :].bitcast(F32R))

                sps = pp.tile([128, QC], F32, tag="s", bufs=2,
                              name=f"sps_{c}_{kt}")
                for oi in range(NO):
                    nc.tensor.matmul(
                        sps, ktile[:, oi, :], qt[oi],
                        start=(oi == 0), stop=(oi == NO - 1))
                di = kt - (nkt - 4)
                if di >= 0:
                    nc.vector.tensor_add(sps, sps, masks[di])
                pt = sb.tile([128, QC], F32R, tag="pt", bufs=4,
                             name=f"pt_{c}_{kt}")
                nc.scalar.activation(pt, sps, EXP, scale=SCALE)

                for s in range(4):
                    nc.tensor.matmul(
                        av[s], pt[:, s * 128:(s + 1) * 128], vt,
                        start=(kt == 0), stop=(kt == nkt - 1))
                nc.tensor.matmul(
                    lps, ones, pt, start=(kt == 0), stop=(kt == nkt - 1))

            for s in range(4):
                ot = sb.tile([128, OV], F32, tag="ot", bufs=3,
                             name=f"ot_{c}_{s}")
                nc.scalar.copy(ot, av[s])
                nc.sync.dma_start(out3[c * 4 + s], ot)
            lt = sb.tile([1, QC], F32, tag="lt", bufs=2, name=f"lt_{c}")
            nc.scalar.copy(lt, lps)
            nc.sync.dma_start(l_out[c:c + 1, :], lt)


_NC_CACHE = None


def build_program():
    global _NC_CACHE
    if _NC_CACHE is not None:
        return _NC_CACHE
    nc = bacc.Bacc("TRN2", target_bir_lowering=False, debug=False)
    xT = nc.dram_tensor("xT", [E, N], F32, kind="ExternalInput").ap()
    WqT = nc.dram_tensor("WqT", [E, O], F32, kind="ExternalInput").ap()
    WkT = nc.dram_tensor("WkT", [E, O], F32, kind="ExternalInput").ap()
    WvT = nc.dram_tensor("WvT", [E, OV], F32, kind="ExternalInput").ap()
    out_u = nc.dram_tensor("out_u", [N, OV], F32, kind="ExternalOutput").ap()
    l_out = nc.dram_tensor("l_out", [NCHUNK, QC], F32, kind="ExternalOutput").ap()
    KT_d = nc.dram_tensor("KT_d", [O, N], F32, kind="Internal").ap()
    V_d = nc.dram_tensor("V_d", [N, OV], F32, kind="Internal").ap()
    with tile.TileContext(nc) as tc:
        _emit(nc, tc, xT, WqT, WkT, WvT, KT_d, V_d, out_u, l_out)
    nc.compile()
    _NC_CACHE = nc
    return nc


def make_in_maps(x, Wq, Wk, Wv):
    x = np.asarray(x, np.float32)
    WqT = np.ascontiguousarray(np.asarray(Wq, np.float32).T)
    WkT = np.ascontiguousarray(np.asarray(Wk, np.float32).T)
    WvT_full = np.ascontiguousarray(np.asarray(Wv, np.float32).T)
    in_maps = []
    for c in range(8):
        b, h = divmod(c, 2)
        in_maps.append({
            "xT": np.ascontiguousarray(x[b].T),
            "WqT": WqT,
            "WkT": WkT,
            "WvT": np.ascontiguousarray(WvT_full[:, h * OV:(h + 1) * OV]),
        })
    return in_maps


def gather_out(results):
    out = np.empty((B, N, O), np.float32)
    for c in range(8):
        b, h = divmod(c, 2)
        ou = results[c]["out_u"].astype(np.float64)
        l = results[c]["l_out"].astype(np.float64).reshape(N, 1)
        out[b, :, h * OV:(h + 1) * OV] = (ou / l).astype(np.float32)
    return out


def kernel(x, Wq, Wk, Wv, **run_kwargs):
    nc = build_program()
    in_maps = make_in_maps(x, Wq, Wk, Wv)
    res = run_bass_kernel_spmd(nc, in_maps, core_ids=list(range(8)),
                               **run_kwargs)
    out = gather_out(res.results)
    if run_kwargs:
        return out, res
    return out


# revision 3
# speedup vs baseline: 5870.7913x; 5870.7913x over previous
"""Causal attention kernel for Trainium2, 8-core SPMD (final).
Interleaved-key split:

8 cores = 4 batches x 2 key-shards.  Core (b, h) handles key tiles
kt === h (mod 2) of batch b (2048 keys) but ALL 4096 queries, producing
unnormalized partial attention sums + partial softmax denominators;
the host merges: out = (avA + avB) / (lA + lB).  This halves the
scores/AV/exp work vs an output-column split; only the Q projection is
duplicated across the pair.

All matmuls f32r (1 cycle/row).  No max-subtraction in softmax
(|scores| <~ 3 by construction), so partials merge exactly.

Per-core causal structure: for query chunk c (512 q), local key tiles
kt' = 0..2c+1 are active; the last two (kt'=2c, 2c+1) are diagonal and
get additive masks.  Masks depend on the shard h, so they are passed
as per-core INPUT data (mask_in[2,128,512]) keeping the program
identical across cores.
"""
import numpy as np

import concourse.bacc as bacc
import concourse.mybir as mybir
import concourse.tile as tile
from concourse.bass_utils import run_bass_kernel_spmd

F32 = mybir.dt.float32
F32R = mybir.dt.float32r
EXP = mybir.ActivationFunctionType.Exp

B, N, E, O = 4, 4096, 1024, 1024
NE, NO = E // 128, O // 128
NK = N // 2                          # local keys per core (2048)
NKT_L = NK // 128                    # 16 local k-tiles
QC = 512
NCHUNK = N // QC                     # 8
SCALE = 1.0 / 32.0
MASKVAL = -1e9


def _emit(nc, tc, xT, xkT, WqT, WkT, WvT, mask_in, KT_d, out_u, l_out):
    xT3 = xT.rearrange("(a p) n -> a p n", p=128)
    xk3 = xkT.rearrange("(a p) n -> a p n", p=128)
    WqT3 = WqT.rearrange("(a p) n -> a p n", p=128)
    WkT3 = WkT.rearrange("(a p) n -> a p n", p=128)
    WvT3 = WvT.rearrange("(a p) n -> a p n", p=128)
    KT_v = KT_d.rearrange("(a p) n -> p a n", p=128)
    out3 = out_u.rearrange("(a p) n -> a p n", p=128)

    # outer pool: resident V tiles + cross-phase weights/masks
    from contextlib import ExitStack
    _stk = ExitStack()
    outer = _stk.enter_context(tc.tile_pool(name="outer", bufs=1))
    vres = [outer.tile([128, O], F32R, tag=f"vres{k}", name=f"vres_{k}")
            for k in range(NKT_L)]
    wq, masks = [], []
    # ---------------- phase 1: K^T (local keys) and V projections ------------
    with tc.tile_pool(name="p1", bufs=1) as sb, \
         tc.tile_pool(name="p1p", bufs=1, space="PSUM") as pp:
        NH2 = NK // 2
        wk, wv = [], []
        for e in range(NE):
            wkt = sb.tile([128, O], F32R, tag=f"wk{e}", name=f"wk_{e}")
            nc.sync.dma_start(wkt, WkT3[e].bitcast(F32R))
            wk.append(wkt)
            wvt = sb.tile([128, O], F32R, tag=f"wv{e}", name=f"wv_{e}")
            nc.sync.dma_start(wvt, WvT3[e].bitcast(F32R))
            wv.append(wvt)
        for e in range(NE):
            wqt = outer.tile([128, O], F32R, tag=f"wq{e}", name=f"wq_{e}")
            nc.sync.dma_start(wqt, WqT3[e].bitcast(F32R))
            wq.append(wqt)
        for i in range(2):
            m = outer.tile([128, QC], F32, tag=f"mask{i}", name=f"mask_{i}")
            nc.sync.dma_start(m, mask_in[i])
            masks.append(m)
        for half in range(2):
            xe = []
            for e in range(NE):
                xet = sb.tile([128, NH2], F32R, tag=f"xe{e}",
                              name=f"xe_{half}_{e}")
                nc.sync.dma_start(
                    xet, xk3[e][:, half * NH2:(half + 1) * NH2].bitcast(F32R))
                xe.append(xet)
            for o in range(NO):
                pk = [pp.tile([128, 512], F32, tag="pp", bufs=8,
                              name=f"pk_{half}_{o}_{kc}")
                      for kc in range(NH2 // 512)]
                for e in range(NE):
                    for kc in range(NH2 // 512):
                        nc.tensor.matmul(
                            pk[kc], wk[e][:, o * 128:(o + 1) * 128],
                            xe[e][:, kc * 512:(kc + 1) * 512],
                            start=(e == 0), stop=(e == NE - 1))
                for kc in range(NH2 // 512):
                    ksb = sb.tile([128, 512], F32, tag="ksb", bufs=3,
                                  name=f"ksb_{half}_{o}_{kc}")
                    nc.scalar.copy(ksb, pk[kc])
                    col = half * NH2 + kc * 512
                    nc.sync.dma_start(
                        KT_d[o * 128:(o + 1) * 128, col:col + 512], ksb)
            for ns in range(NH2 // 128):
                gk = half * (NH2 // 128) + ns
                for ovc in range(2):
                    pv = pp.tile([128, 512], F32, tag="pp", bufs=8,
                                 name=f"pv_{half}_{ns}_{ovc}")
                    for e in range(NE):
                        nc.tensor.matmul(
                            pv, xe[e][:, ns * 128:(ns + 1) * 128],
                            wv[e][:, ovc * 512:(ovc + 1) * 512],
                            start=(e == 0), stop=(e == NE - 1))
                    nc.scalar.copy(
                        vres[gk][:, ovc * 512:(ovc + 1) * 512], pv)

    # ---------------- phase 2: attention ----------------
    with tc.tile_pool(name="p2", bufs=1) as sb, \
         tc.tile_pool(name="p2p", bufs=1, space="PSUM") as pp:
        ones = sb.tile([128, 1], F32, tag="ones", name="ones")
        nc.gpsimd.memset(ones, 1.0)

        for c in range(NCHUNK):
            nkt = 2 * c + 2
            xc = []
            for e in range(NE):
                xct = sb.tile([128, QC], F32R, tag=f"xc{e}", bufs=1,
                              name=f"xc_{c}_{e}")
                nc.sync.dma_start(
                    xct, xT3[e][:, c * QC:(c + 1) * QC].bitcast(F32R))
                xc.append(xct)
            qt = []
            for oi in range(NO):
                qps = pp.tile([128, QC], F32, tag="avqp", bufs=4,
                              name=f"qps_{c}_{oi}")
                for e in range(NE):
                    nc.tensor.matmul(
                        qps, wq[e][:, oi * 128:(oi + 1) * 128], xc[e],
                        start=(e == 0), stop=(e == NE - 1))
                qtt = sb.tile([128, QC], F32R, tag=f"qt{oi}", bufs=1,
                              name=f"qt_{c}_{oi}")
                nc.scalar.copy(qtt, qps)
                qt.append(qtt)

            lacc = sb.tile([128, QC], F32, tag="lacc", bufs=2,
                           name=f"lacc_{c}")
            lps = pp.tile([1, QC], F32, tag="l", bufs=1, name=f"lps_{c}")

            # scores + exp for the whole chunk; pt persists in SBUF
            pts = []
            for kt in range(nkt):
                ktile = sb.tile([128, NO, 128], F32R, tag="ktile", bufs=3,
                                name=f"ktile_{c}_{kt}")
                nc.sync.dma_start(
                    ktile, KT_v[:, :, kt * 128:(kt + 1) * 128].bitcast(F32R))

                sps = pp.tile([128, QC], F32, tag="s", bufs=3,
                              name=f"sps_{c}_{kt}")
                for oi in range(NO):
                    nc.tensor.matmul(
                        sps, ktile[:, oi, :], qt[oi],
                        start=(oi == 0), stop=(oi == NO - 1))
                di = kt - (nkt - 2)
                if di >= 0:
                    nc.vector.tensor_add(sps, sps, masks[di])
                pt = sb.tile([128, QC], F32R, tag=f"pt{kt}", bufs=1,
                             name=f"pt_{c}_{kt}")
                nc.scalar.activation(pt, sps, EXP, scale=SCALE)
                pts.append(pt)
                if kt == 0:
                    nc.vector.tensor_copy(lacc, pt.bitcast(F32))
                else:
                    nc.vector.tensor_add(lacc, lacc, pt.bitcast(F32))

            # AV: two passes over the kept pt/vt tiles, PSUM-accumulated
            for ovc in range(2):
                av = [pp.tile([128, 512], F32, tag="avqp", bufs=4,
                              name=f"av_{c}_{ovc}_{s}") for s in range(4)]
                for kt in range(nkt):
                    for s in range(4):
                        nc.tensor.matmul(
                            av[s], pts[kt][:, s * 128:(s + 1) * 128],
                            vres[kt][:, ovc * 512:(ovc + 1) * 512],
                            start=(kt == 0), stop=(kt == nkt - 1))
                for s in range(4):
                    ot = sb.tile([128, 512], F32, tag="ot", bufs=2,
                                 name=f"ot_{c}_{ovc}_{s}")
                    nc.scalar.copy(ot, av[s])
                    nc.sync.dma_start(
                        out3[c * 4 + s][:, ovc * 512:(ovc + 1) * 512], ot)

            nc.tensor.matmul(lps, ones, lacc, start=True, stop=True)
            lt = sb.tile([1, QC], F32, tag="lt", bufs=2, name=f"lt_{c}")
            nc.scalar.copy(lt, lps)
            nc.sync.dma_start(l_out[c:c + 1, :], lt)
    _stk.close()


_NC_CACHE = None


def build_program():
    global _NC_CACHE
    if _NC_CACHE is not None:
        return _NC_CACHE
    nc = bacc.Bacc("TRN2", target_bir_lowering=False, debug=False)
    xT = nc.dram_tensor("xT", [E, N], F32, kind="ExternalInput").ap()
    xkT = nc.dram_tensor("xkT", [E, NK], F32, kind="ExternalInput").ap()
    WqT = nc.dram_tensor("WqT", [E, O], F32, kind="ExternalInput").ap()
    WkT = nc.dram_tensor("WkT", [E, O], F32, kind="ExternalInput").ap()
    WvT = nc.dram_tensor("WvT", [E, O], F32, kind="ExternalInput").ap()
    mask_in = nc.dram_tensor("mask_in", [2, 128, QC], F32,
                             kind="ExternalInput").ap()
    out_u = nc.dram_tensor("out_u", [N, O], F32, kind="ExternalOutput").ap()
    l_out = nc.dram_tensor("l_out", [NCHUNK, QC], F32,
                           kind="ExternalOutput").ap()
    KT_d = nc.dram_tensor("KT_d", [O, NK], F32, kind="Internal").ap()
    with tile.TileContext(nc) as tc:
        _emit(nc, tc, xT, xkT, WqT, WkT, WvT, mask_in, KT_d, out_u, l_out)
    nc.compile()
    _NC_CACHE = nc
    return nc


def make_in_maps(x, Wq, Wk, Wv):
    x = np.asarray(x, np.float32)
    WqT = np.ascontiguousarray(np.asarray(Wq, np.float32).T)
    WkT = np.ascontiguousarray(np.asarray(Wk, np.float32).T)
    WvT = np.ascontiguousarray(np.asarray(Wv, np.float32).T)
    kk = np.arange(128)[:, None]
    qq = np.arange(QC)[None, :]
    in_maps = []
    for c in range(8):
        b, h = divmod(c, 2)
        xb = x[b]
        xk = xb.reshape(N // 128, 128, E)[h::2].reshape(NK, E)
        masks = np.stack([
            np.where(qq >= (2 * i + h) * 128 + kk, 0.0, MASKVAL)
            for i in range(2)
        ]).astype(np.float32)
        in_maps.append({
            "xT": np.ascontiguousarray(xb.T),
            "xkT": np.ascontiguousarray(xk.T),
            "WqT": WqT,
            "WkT": WkT,
            "WvT": WvT,
            "mask_in": masks,
        })
    return in_maps


def gather_out(results):
    out = np.empty((B, N, O), np.float32)
    for b in range(B):
        a0 = results[2 * b]["out_u"].astype(np.float64)
        a1 = results[2 * b + 1]["out_u"].astype(np.float64)
        l0 = results[2 * b]["l_out"].astype(np.float64).reshape(N, 1)
        l1 = results[2 * b + 1]["l_out"].astype(np.float64).reshape(N, 1)
        out[b] = ((a0 + a1) / (l0 + l1)).astype(np.float32)
    return out


def kernel(x, Wq, Wk, Wv, **run_kwargs):
    nc = build_program()
    in_maps = make_in_maps(x, Wq, Wk, Wv)
    res = run_bass_kernel_spmd(nc, in_maps, core_ids=list(range(8)),
                               **run_kwargs)
    out = gather_out(res.results)
    if run_kwargs:
        return out, res
    return out
